# revision 92
# baseline (speedup 1.0000x reference)
"""Bernstein flow density kernel for 8x TRN2 NeuronCores.

Math (per sample n):
  density(n) = prod_i [ phi_i[n,15] + sum_m tf_i[n,m] * psi_i[n,m] ]
  tf_i = cond_i @ c_alpha_i,  cond_i = B_0 (x) ... (x) B_{i-1}  (row-wise Kron)
Bernstein bases sum to 1, so cond_i is a marginal of cond_5 [N,1024] and all
six matmuls merge into ONE with a 96-wide W.  Both the binomial scaling and
the coefficient finite-difference fold into W host-side:
  factor_i = sum_{m=0..15} (ca_m - ca_{m-1}) * kap_m * x^m (1-x)^(15-m)
  W96[c, i*16+m] = kap_m * (ca_i[pre,m] - ca_i[pre,m-1]),  ca_{-1}=0, ca_15=1
  tf96 = cond_5 @ W96;  vtab[n, i,m] = x_i^m (1-x_i)^(15-m)
  factor_i = sum_m tf96 * vtab  ->  density = prod_i factor_i

Per core (8192 samples, p-major: local n = p*64 + s), per group g of 512
samples (s = 4g..4g+3):
  1. DVE/Pool build deg-3 tables, k3/q34, then cond tiles [128,1024] in bf16
  2. XBAR DMA transpose (SBUF->SBUF, 2-byte) -> ctb [128(c%128), 8, 512] bf16
  3. PE: 8 accumulating bf16 matmuls vs W96 chunks -> tf96^T [96,512] psum
     (moving dim 512 -> 1 cycle/row), Act stashes to SBUF
  4. PE transposes vtab natural [128,96] -> v^T [96,512] (bf16 identity),
     eprod = tf^T (*) v^T, E-matmul reduces (i,m)->i -> factors [6,512],
     tiny back-transpose -> natural [128, s, 6]; final 3-op product -> dens
"""

import math
import sys

import numpy as np

sys.path.insert(0, "/opt/trn_rl_repo")

import concourse.bacc as bacc  # noqa: E402
import concourse.bass as bass  # noqa: E402
import concourse.tile as tile  # noqa: E402
from concourse import mybir  # noqa: E402
from concourse.bass_utils import run_bass_kernel_spmd  # noqa: E402

N = 65536
DIM = 6
NCORES = 8
NC = N // NCORES          # 8192 samples per core
P = 128
S = NC // P               # 64 samples per partition
NT = 4                    # s-tiles per group
NG = S // NT              # 16 groups (512 samples each)
NB = NT * P               # 512 samples per group
CDIM = 1024               # cond_5 width
KCH = CDIM // P           # 8 contraction chunks
M96 = 96                  # 6 dims * 16 coeffs (m=15 carries phi)

F32 = mybir.dt.float32
F32R = mybir.dt.float32r
BF16 = mybir.dt.bfloat16
MUL = mybir.AluOpType.mult
ADD = mybir.AluOpType.add
SUB = mybir.AluOpType.subtract
AF = mybir.ActivationFunctionType

_CACHE = {}


def _ap(a, off_elems, dims):
    """AP over slice a with replaced free dims; dims = [[step,count],...]."""
    return bass.AP(tensor=a.tensor, offset=a.offset + off_elems, ap=[a.ap[0]] + dims)


def _build_nc():
    nc = bacc.Bacc(target_bir_lowering=False, trn_type="TRN2")

    xr = nc.dram_tensor("xr", [P, S, DIM], F32, kind="ExternalInput")
    wmat = nc.dram_tensor("wmat", [CDIM, M96], F32, kind="ExternalInput")
    ident = nc.dram_tensor("ident", [P, P], F32, kind="ExternalInput")
    e96r = nc.dram_tensor("e96r", [M96, DIM], F32, kind="ExternalInput")
    dens_out = nc.dram_tensor("dens", [P, S], F32, kind="ExternalOutput")

    with tile.TileContext(nc) as tc:
        with (
            tc.tile_pool(name="singles", bufs=1) as singles,
            tc.tile_pool(name="bigs", bufs=1) as bigs,
            tc.tile_pool(name="cond", bufs=6) as condp,
            tc.tile_pool(name="ctb", bufs=2) as ctbp,
            tc.tile_pool(name="pows", bufs=1) as powp,
            tc.tile_pool(name="eprodp", bufs=2) as eprodp,
            tc.tile_pool(name="ps_tf", bufs=2, space="PSUM") as ps_tf,
            tc.tile_pool(name="ps_psi", bufs=2, space="PSUM") as ps_psi,
            tc.tile_pool(name="ps_ct", bufs=1, space="PSUM") as ps_ct,
            tc.tile_pool(name="ps_fac", bufs=2, space="PSUM") as ps_fac,
            tc.tile_pool(name="ps_ft", bufs=1, space="PSUM") as ps_ft,
        ):
            # ---- constants / inputs ----
            xin = singles.tile([P, S, DIM], F32)
            nc.sync.dma_start(out=xin[:, :, :], in_=xr[:, :, :])
            idf = singles.tile([P, P], F32)
            nc.sync.dma_start(out=idf[:, :], in_=ident[:, :])
            wf = singles.tile([P, KCH, M96], F32)
            nc.sync.dma_start(
                out=wf[:, :, :],
                in_=bass.AP(tensor=wmat[:, :].tensor, offset=0,
                            ap=[[M96, P], [P * M96, KCH], [1, M96]]),
            )
            # bf16 casts: W chunks + identity (rhs of transposes -> 1 cyc/row)
            # idnr: f32r identity for f32r-lhsT transposes (walrus requires
            # fp32/f32r operands to match dtypes exactly)
            wsb = singles.tile([P, KCH, M96], BF16)
            nc.vector.tensor_copy(out=wsb[:, :, :], in_=wf[:, :, :])
            idn = singles.tile([P, P], BF16)
            nc.vector.tensor_copy(out=idn[:, :], in_=idf[:, :])
            idnr = singles.tile([P, P], F32R)
            nc.vector.tensor_copy(out=idnr[:, :], in_=idf[:, :])
            # E96 [96, 6] f32r: block one-hot reducing (i,m)->i
            e96f = singles.tile([M96, DIM], F32)
            nc.sync.dma_start(out=e96f[:, :], in_=e96r[:, :])
            e96 = singles.tile([M96, DIM], F32R)
            nc.vector.tensor_copy(out=e96[:, :], in_=e96f[:, :])

            xa = xin[:, :, :]
            NJ = 5

            # ---- stage A/B: powers + deg-3 tables Bbig[p, s, j, a] ----
            omx = singles.tile([P, S, DIM], F32)
            x2 = singles.tile([P, S, DIM], F32)
            x3 = singles.tile([P, S, DIM], F32)
            omx2 = singles.tile([P, S, DIM], F32)
            omx3 = singles.tile([P, S, DIM], F32)
            Bbig = singles.tile([P, S, NJ, 4], F32)

            def emit_stages_ab():
                nc.vector.tensor_scalar(
                    out=omx[:, :, :], in0=xa, scalar1=-1.0, scalar2=1.0,
                    op0=MUL, op1=ADD)
                nc.scalar.square(out=x2[:, :, :], in_=xa)
                nc.scalar.square(out=omx2[:, :, :], in_=omx[:, :, :])
                nc.vector.tensor_tensor(
                    out=x3[:, :, :], in0=x2[:, :, :], in1=xa, op=MUL)
                nc.gpsimd.tensor_tensor(
                    out=omx3[:, :, :], in0=omx2[:, :, :], in1=omx[:, :, :], op=MUL)
                for (a, src, scl, other) in (
                    (0, omx3, None, None),
                    (1, xin, 3.0, omx2),
                    (2, x2, 3.0, omx),
                    (3, x3, None, None),
                ):
                    src_ap = _ap(src[:, :, :], 0, [[DIM, S], [1, NJ]])
                    out_ap = _ap(Bbig[:, :, :, :], a, [[4 * NJ, S], [4, NJ]])
                    if scl is None:
                        nc.vector.tensor_copy(out=out_ap, in_=src_ap)
                    else:
                        nc.vector.scalar_tensor_tensor(
                            out=out_ap, in0=src_ap, scalar=scl,
                            in1=_ap(other[:, :, :], 0, [[DIM, S], [1, NJ]]),
                            op0=MUL, op1=MUL)

            def emit_prologue_b0():
                """Fast-path mini A/B for group 0 (s in [0,4)), all on DVE."""
                pomx = singles.tile([P, NT, DIM], F32)
                pw = singles.tile([P, 4, NT, DIM], F32)  # x2, omx2, x3, omx3
                bb0 = singles.tile([P, NT, NJ, 4], F32)
                xa0 = _ap(xin[:, :, :], 0, [[DIM, NT], [1, DIM]])
                pa = [_ap(pw[:, :, :, :], q * NT * DIM, [[DIM, NT], [1, DIM]])
                      for q in range(4)]
                oa = _ap(pomx[:, :, :], 0, [[DIM, NT], [1, DIM]])
                nc.vector.tensor_scalar(
                    out=oa, in0=xa0, scalar1=-1.0, scalar2=1.0, op0=MUL, op1=ADD)
                nc.vector.tensor_tensor(out=pa[0], in0=xa0, in1=xa0, op=MUL)
                nc.vector.tensor_tensor(out=pa[1], in0=oa, in1=oa, op=MUL)
                nc.vector.tensor_tensor(out=pa[2], in0=pa[0], in1=xa0, op=MUL)
                nc.vector.tensor_tensor(out=pa[3], in0=pa[1], in1=oa, op=MUL)
                for (a, src, scl, other) in (
                    (0, pa[3], None, None),
                    (1, xa0, 3.0, pa[1]),
                    (2, pa[0], 3.0, oa),
                    (3, pa[2], None, None),
                ):
                    src_ap = bass.AP(tensor=src.tensor, offset=src.offset,
                                     ap=[src.ap[0], [DIM, NT], [1, NJ]])
                    out_ap = _ap(bb0[:, :, :, :], a, [[4 * NJ, NT], [4, NJ]])
                    if scl is None:
                        nc.vector.tensor_copy(out=out_ap, in_=src_ap)
                    else:
                        oth = bass.AP(tensor=other.tensor, offset=other.offset,
                                      ap=[other.ap[0], [DIM, NT], [1, NJ]])
                        nc.vector.scalar_tensor_tensor(
                            out=out_ap, in0=src_ap, scalar=scl, in1=oth,
                            op0=MUL, op1=MUL)
                return bb0

            # ---- vtab natural [P, S, 96] (f32r for PE transpose) ----
            vtab = bigs.tile([P, S, DIM, 16], BF16)
            t1f = bigs.tile([P, S, 4], F32)
            dq = bigs.tile([P, S], F32)
            tfsb = bigs.tile([M96, NG, NB], F32)   # stashed tf96^T per group
            facn = bigs.tile([P, S, DIM], F32R)    # factors back in natural

            SD = S * DIM  # 384: ladders for all 6 dims at once
            px = powp.tile([P, 16, SD], F32, tag="px")
            pq = powp.tile([P, 16, SD], F32, tag="pq")

            def ladder_chunk(i):
                """Emit chunk i of the px (DVE) + pq (Pool) power ladders."""
                for (tbl, base, eng) in ((px, xin, nc.vector), (pq, omx, nc.gpsimd)):
                    t1 = tbl[:, :, :]
                    if i == 0:
                        eng.memset(tbl[:, 0, :], 1.0)
                        eng.tensor_copy(
                            out=tbl[:, 1, :], in_=_ap(base[:, :, :], 0, [[1, SD]]))
                        eng.tensor_tensor(
                            out=tbl[:, 2, :], in0=t1[:, 1, :], in1=t1[:, 1, :],
                            op=MUL)
                    elif i == 1:
                        eng.tensor_tensor(
                            out=_ap(t1, 3 * SD, [[1, 2 * SD]]),
                            in0=_ap(t1, SD, [[1, 2 * SD]]),
                            in1=_ap(t1, 2 * SD, [[0, 2], [1, SD]]), op=MUL)
                    elif i == 2:
                        eng.tensor_tensor(
                            out=_ap(t1, 5 * SD, [[1, 4 * SD]]),
                            in0=_ap(t1, SD, [[1, 4 * SD]]),
                            in1=_ap(t1, 4 * SD, [[0, 4], [1, SD]]), op=MUL)
                    else:
                        eng.tensor_tensor(
                            out=_ap(t1, 9 * SD, [[1, 7 * SD]]),
                            in0=_ap(t1, SD, [[1, 7 * SD]]),
                            in1=_ap(t1, 8 * SD, [[0, 7], [1, SD]]), op=MUL)

            def emit_vtab(j):
                # vtab[:, s, j, m] = px[m, s, j] * pq[15-m, s, j]
                eng = nc.gpsimd if j % 2 == 0 else nc.vector
                eng.tensor_tensor(
                    out=_ap(vtab[:, :, :, :], j * 16, [[1, 16], [DIM * 16, S]]),
                    in0=_ap(px[:, :, :], j, [[SD, 16], [DIM, S]]),
                    in1=_ap(pq[:, :, :], 15 * SD + j, [[-SD, 16], [DIM, S]]), op=MUL)

            # ---- stage D: cond -> XBAR transpose -> matmul -> stash ----
            def emit_group_mm(g, bb=None, gb=None):
                if bb is None:
                    bb = Bbig[:, :, :, :]
                    gb = g * NT * NJ * 4
                TS = NJ * 4
                k2g = condp.tile([P, NT, 16], F32, tag="k2")
                k3g = condp.tile([P, NT, 64], F32, tag="k3")
                q34g = condp.tile([P, NT, 16], F32, tag="q34")
                ea = nc.vector if g % 2 == 0 else nc.gpsimd
                eb = nc.gpsimd if g % 2 == 0 else nc.vector
                ea.tensor_tensor(
                    out=k2g[:, :, :],
                    in0=_ap(bb, gb + 0, [[TS, NT], [1, 4], [0, 4]]),
                    in1=_ap(bb, gb + 4, [[TS, NT], [0, 4], [1, 4]]), op=MUL)
                ea.tensor_tensor(
                    out=k3g[:, :, :],
                    in0=_ap(k2g[:, :, :], 0, [[16, NT], [1, 16], [0, 4]]),
                    in1=_ap(bb, gb + 8, [[TS, NT], [0, 16], [1, 4]]), op=MUL)
                eb.tensor_tensor(
                    out=q34g[:, :, :],
                    in0=_ap(bb, gb + 12, [[TS, NT], [1, 4], [0, 4]]),
                    in1=_ap(bb, gb + 16, [[TS, NT], [0, 4], [1, 4]]), op=MUL)
                ctb = ctbp.tile([P, KCH, NB], BF16, tag="ctb")
                for t in range(NT):
                    cnd = condp.tile([P, CDIM], BF16, tag=f"cond{t}")
                    idx = 4 * g + t
                    eng = nc.vector if (idx * 25) % 64 < 25 else nc.gpsimd
                    eng.tensor_tensor(
                        out=cnd[:, :],
                        in0=_ap(k3g[:, :, :], t * 64, [[1, 64], [0, 16]]),
                        in1=_ap(q34g[:, :, :], t * 16, [[0, 64], [1, 16]]), op=MUL)
                    if t < 3:
                        # XBAR: ctb[c%128, c//128, t*128+p] = cnd[p, c]
                        nc.sync.dma_start_transpose(
                            out=_ap(ctb[:, :, :], t * P, [[NB, KCH], [1, P]]),
                            in_=cnd[:, :])
                    else:
                        # PE transpose (8 blocks) + Act copy to offload XBAR
                        pct = ps_ct.tile([P, CDIM], BF16, tag="pct")
                        for k in range(KCH):
                            nc.tensor.matmul(
                                out=_ap(pct[:, :], k * P, [[1, P]]),
                                lhsT=cnd[:, k * P:(k + 1) * P],
                                rhs=idn[:, :],
                                is_transpose=True, start=True, stop=True,
                                skip_group_check=True)
                        nc.scalar.copy(
                            out=_ap(ctb[:, :, :], t * P, [[NB, KCH], [1, P]]),
                            in_=pct[:, :])
                tfps = ps_tf.tile([M96, NB], F32, tag="tf")
                for k in range(KCH):
                    nc.tensor.matmul(
                        out=tfps[:, :],
                        lhsT=wsb[:, k, :],
                        rhs=ctb[:, k, :],
                        start=(k == 0), stop=(k == KCH - 1))
                if g < NG - 2:
                    nc.scalar.copy(
                        out=_ap(tfsb[:, :, :], g * NB, [[1, NB]]),
                        in_=tfps[:, :])
                else:
                    nc.vector.tensor_copy(
                        out=_ap(tfsb[:, :, :], g * NB, [[1, NB]]),
                        in_=tfps[:, :])

            # ---- stage E: transposed combine per group ----
            def emit_group_combine(g):
                ppsi = ps_psi.tile([M96, NB], BF16, tag="ppsi")
                for t in range(NT):
                    nc.tensor.matmul(
                        out=_ap(ppsi[:, :], t * P, [[1, P]]),
                        lhsT=_ap(vtab[:, :, :, :], (4 * g + t) * M96, [[1, M96]]),
                        rhs=idn[:, :],
                        is_transpose=True, start=True, stop=True,
                        skip_group_check=True)
                psiT = eprodp.tile([M96, NB], BF16, tag="psiT")
                if g % 3 == 0:
                    nc.scalar.copy(out=psiT[:, :], in_=ppsi[:, :])
                else:
                    nc.vector.tensor_copy(out=psiT[:, :], in_=ppsi[:, :])
                eprod = eprodp.tile([M96, NB], F32R, tag="eprod")
                nc.gpsimd.tensor_tensor(
                    out=eprod[:, :],
                    in0=_ap(tfsb[:, :, :], g * NB, [[1, NB]]),
                    in1=psiT[:, :], op=MUL)
                fac = ps_fac.tile([DIM, NB], F32, tag="fac")
                nc.tensor.matmul(
                    out=fac[:, :], lhsT=e96[:, :], rhs=eprod[:, :],
                    start=True, stop=True)
                facsb = eprodp.tile([DIM, NB], F32R, tag="facsb")
                if g < NG - 3:
                    nc.scalar.copy(out=facsb[:, :], in_=fac[:, :])
                else:
                    nc.vector.tensor_copy(out=facsb[:, :], in_=fac[:, :])
                facT = ps_ft.tile([P, NT, DIM], F32R, tag="facT")
                for t in range(NT):
                    nc.tensor.matmul(
                        out=facT[:, t, :],
                        lhsT=_ap(facsb[:, :], t * P, [[1, P]]),
                        rhs=idnr[:DIM, :DIM],
                        is_transpose=True, start=True, stop=True,
                        skip_group_check=True)
                nc.scalar.copy(
                    out=_ap(facn[:, :, :], 4 * g * DIM, [[1, NT * DIM]]),
                    in_=facT[:, :, :])
                # incremental density product for this group's s-range
                eng3 = nc.vector if g % 2 == 0 else nc.gpsimd
                eng3.tensor_tensor(
                    out=_ap(t1f[:, :, :], 4 * g * 4, [[4, NT], [1, 3]]),
                    in0=_ap(facn[:, :, :], 4 * g * DIM, [[DIM, NT], [2, 3]]),
                    in1=_ap(facn[:, :, :], 4 * g * DIM + 1, [[DIM, NT], [2, 3]]),
                    op=MUL)
                eng3.tensor_tensor(
                    out=_ap(t1f[:, :, :], 4 * g * 4 + 3, [[4, NT]]),
                    in0=_ap(t1f[:, :, :], 4 * g * 4, [[4, NT]]),
                    in1=_ap(t1f[:, :, :], 4 * g * 4 + 1, [[4, NT]]), op=MUL)
                eng3.tensor_tensor(
                    out=_ap(dq[:, :], 4 * g, [[1, NT]]),
                    in0=_ap(t1f[:, :, :], 4 * g * 4 + 3, [[4, NT]]),
                    in1=_ap(t1f[:, :, :], 4 * g * 4 + 2, [[4, NT]]), op=MUL)
                if g == 7:
                    nc.sync.dma_start(out=dens_out[:, :32], in_=dq[:, :32])

            # ---- schedule: group-0 fast path, then full tables; ladders/vtab
            # interleaved into groups 1..6, combines trail mm by K groups ----
            K = 6
            emit_stages_ab()
            for g in range(0, NG):
                emit_group_mm(g)
                if g < 4:
                    ladder_chunk(g)
                elif g == 4:
                    emit_vtab(0), emit_vtab(1), emit_vtab(2)
                elif g == 5:
                    emit_vtab(3), emit_vtab(4), emit_vtab(5)
                if g >= K:
                    emit_group_combine(g - K)
            for g in range(NG - K, NG):
                emit_group_combine(g)

            # ---- final: second half of density already computed per group
            nc.sync.dma_start(out=dens_out[:, 32:], in_=dq[:, 32:])

    nc.finalize()
    return nc


def _softplus64(v):
    return np.logaddexp(0.0, v)


def _host_w(As):
    """W96 [1024, 96]: col i*16+m = kap_m * (ca_m - ca_{m-1}), ca_{-1}=0,
    ca_15=1 (binomial scaling + coeff finite-difference folded in)."""
    kap = 16.0 * np.array([math.comb(15, m) for m in range(16)], dtype=np.float64)
    cols = []
    for i in range(DIM):
        c = np.cumsum(_softplus64(As[i].astype(np.float64)), axis=1)
        ca = 2.0 * (1.0 / (1.0 + np.exp(-c)) - 0.5)
        rows = ca.shape[0]
        ext = np.concatenate(
            [np.zeros((rows, 1)), ca, np.ones((rows, 1))], axis=1)  # [r, 17]
        blk = kap * (ext[:, 1:] - ext[:, :-1])                      # [r, 16]
        cols.append(np.repeat(blk, 4 ** (5 - i), axis=0))
    return np.concatenate(cols, axis=1).astype(np.float32)


def kernel(**inputs):
    x = np.asarray(inputs["x"], dtype=np.float32)
    As = [np.asarray(inputs[f"A{i}"], dtype=np.float32) for i in range(DIM)]

    if "nc" not in _CACHE:
        _CACHE["nc"] = _build_nc()
    nc = _CACHE["nc"]

    w = _host_w(As)
    idn = np.eye(P, dtype=np.float32)
    e96 = np.zeros((M96, DIM), dtype=np.float32)
    for i in range(DIM):
        e96[i * 16:(i + 1) * 16, i] = 1.0

    in_maps = []
    for c in range(NCORES):
        xc = x[c * NC:(c + 1) * NC].reshape(P, S, DIM)
        in_maps.append({"xr": xc, "wmat": w, "ident": idn, "e96r": e96})

    res = run_bass_kernel_spmd(nc, in_maps, core_ids=list(range(NCORES)))
    outs = [r["dens"].reshape(NC) for r in res.results]
    return np.concatenate(outs, axis=0)


if __name__ == "__main__":
    rng = np.random.default_rng(0)
    ins = {"x": rng.uniform(0, 1, (N, DIM)).astype(np.float32)}
    for i in range(DIM):
        ins[f"A{i}"] = rng.uniform(0, 1, ((4 ** i), 15)).astype(np.float32)
    out = kernel(**ins)
    print(out.shape, out[:4])


# revision 100
# speedup vs baseline: 1.0190x; 1.0190x over previous
"""Bernstein flow density kernel for 8x TRN2 NeuronCores.

Math (per sample n):
  density(n) = prod_i [ phi_i[n,15] + sum_m tf_i[n,m] * psi_i[n,m] ]
  tf_i = cond_i @ c_alpha_i,  cond_i = B_0 (x) ... (x) B_{i-1}  (row-wise Kron)
Bernstein bases sum to 1, so cond_i is a marginal of cond_5 [N,1024] and all
six matmuls merge into ONE with a 96-wide W.  Both the binomial scaling and
the coefficient finite-difference fold into W host-side:
  factor_i = sum_{m=0..15} (ca_m - ca_{m-1}) * kap_m * x^m (1-x)^(15-m)
  W96[c, i*16+m] = kap_m * (ca_i[pre,m] - ca_i[pre,m-1]),  ca_{-1}=0, ca_15=1
  tf96 = cond_5 @ W96;  vtab[n, i,m] = x_i^m (1-x_i)^(15-m)
  factor_i = sum_m tf96 * vtab  ->  density = prod_i factor_i

Per core (8192 samples, p-major: local n = p*64 + s), per group g of 512
samples (s = 4g..4g+3):
  1. DVE/Pool build deg-3 tables, k3/q34, then cond tiles [128,1024] in bf16
  2. XBAR DMA transpose (SBUF->SBUF, 2-byte) -> ctb [128(c%128), 8, 512] bf16
  3. PE: 8 accumulating bf16 matmuls vs W96 chunks -> tf96^T [96,512] psum
     (moving dim 512 -> 1 cycle/row), Act stashes to SBUF
  4. PE transposes vtab natural [128,96] -> v^T [96,512] (bf16 identity),
     eprod = tf^T (*) v^T, E-matmul reduces (i,m)->i -> factors [6,512],
     tiny back-transpose -> natural [128, s, 6]; final 3-op product -> dens
"""

import math
import sys

import numpy as np

sys.path.insert(0, "/opt/trn_rl_repo")

import concourse.bacc as bacc  # noqa: E402
import concourse.bass as bass  # noqa: E402
import concourse.tile as tile  # noqa: E402
from concourse import mybir  # noqa: E402
from concourse.bass_utils import run_bass_kernel_spmd  # noqa: E402

N = 65536
DIM = 6
NCORES = 8
NC = N // NCORES          # 8192 samples per core
P = 128
S = NC // P               # 64 samples per partition
NT = 4                    # s-tiles per group
NG = S // NT              # 16 groups (512 samples each)
NB = NT * P               # 512 samples per group
CDIM = 1024               # cond_5 width
KCH = CDIM // P           # 8 contraction chunks
M96 = 96                  # 6 dims * 16 coeffs (m=15 carries phi)

F32 = mybir.dt.float32
F32R = mybir.dt.float32r
BF16 = mybir.dt.bfloat16
MUL = mybir.AluOpType.mult
ADD = mybir.AluOpType.add
SUB = mybir.AluOpType.subtract
AF = mybir.ActivationFunctionType

_CACHE = {}


def _ap(a, off_elems, dims):
    """AP over slice a with replaced free dims; dims = [[step,count],...]."""
    return bass.AP(tensor=a.tensor, offset=a.offset + off_elems, ap=[a.ap[0]] + dims)


def _build_nc():
    nc = bacc.Bacc(target_bir_lowering=False, trn_type="TRN2")

    xr = nc.dram_tensor("xr", [P, S, DIM], F32, kind="ExternalInput")
    wmat = nc.dram_tensor("wmat", [CDIM, M96], F32, kind="ExternalInput")
    ident = nc.dram_tensor("ident", [P, P], F32, kind="ExternalInput")
    e96r = nc.dram_tensor("e96r", [M96, DIM], F32, kind="ExternalInput")
    dens_out = nc.dram_tensor("dens", [P, S], F32, kind="ExternalOutput")

    with tile.TileContext(nc) as tc:
        with (
            tc.tile_pool(name="singles", bufs=1) as singles,
            tc.tile_pool(name="bigs", bufs=1) as bigs,
            tc.tile_pool(name="cond", bufs=6) as condp,
            tc.tile_pool(name="ctb", bufs=3) as ctbp,
            tc.tile_pool(name="pows", bufs=1) as powp,
            tc.tile_pool(name="eprodp", bufs=2) as eprodp,
            tc.tile_pool(name="ps_tf", bufs=2, space="PSUM") as ps_tf,
            tc.tile_pool(name="ps_psi", bufs=2, space="PSUM") as ps_psi,
            tc.tile_pool(name="ps_ct", bufs=1, space="PSUM") as ps_ct,
            tc.tile_pool(name="ps_fac", bufs=2, space="PSUM") as ps_fac,
            tc.tile_pool(name="ps_ft", bufs=1, space="PSUM") as ps_ft,
        ):
            # ---- constants / inputs ----
            xin = singles.tile([P, S, DIM], F32)
            nc.sync.dma_start(out=xin[:, :, :], in_=xr[:, :, :])
            idf = singles.tile([P, P], F32)
            nc.sync.dma_start(out=idf[:, :], in_=ident[:, :])
            wf = singles.tile([P, KCH, M96], F32)
            nc.sync.dma_start(
                out=wf[:, :, :],
                in_=bass.AP(tensor=wmat[:, :].tensor, offset=0,
                            ap=[[M96, P], [P * M96, KCH], [1, M96]]),
            )
            # bf16 casts: W chunks + identity (rhs of transposes -> 1 cyc/row)
            # idnr: f32r identity for f32r-lhsT transposes (walrus requires
            # fp32/f32r operands to match dtypes exactly)
            wsb = singles.tile([P, KCH, M96], BF16)
            nc.vector.tensor_copy(out=wsb[:, :, :], in_=wf[:, :, :])
            idn = singles.tile([P, P], BF16)
            nc.vector.tensor_copy(out=idn[:, :], in_=idf[:, :])
            idnr = singles.tile([P, P], F32R)
            nc.vector.tensor_copy(out=idnr[:, :], in_=idf[:, :])
            # E96 [96, 6] f32r: block one-hot reducing (i,m)->i
            e96f = singles.tile([M96, DIM], F32)
            nc.sync.dma_start(out=e96f[:, :], in_=e96r[:, :])
            e96 = singles.tile([M96, DIM], BF16)
            nc.vector.tensor_copy(out=e96[:, :], in_=e96f[:, :])

            xa = xin[:, :, :]
            NJ = 5

            # ---- stage A/B: powers + deg-3 tables Bbig[p, s, j, a] ----
            omx = singles.tile([P, S, DIM], F32)
            x2 = singles.tile([P, S, DIM], F32)
            x3 = singles.tile([P, S, DIM], F32)
            omx2 = singles.tile([P, S, DIM], F32)
            omx3 = singles.tile([P, S, DIM], F32)
            Bbig = singles.tile([P, S, NJ, 4], F32)

            def emit_stages_ab():
                nc.vector.tensor_scalar(
                    out=omx[:, :, :], in0=xa, scalar1=-1.0, scalar2=1.0,
                    op0=MUL, op1=ADD)
                nc.scalar.square(out=x2[:, :, :], in_=xa)
                nc.scalar.square(out=omx2[:, :, :], in_=omx[:, :, :])
                nc.vector.tensor_tensor(
                    out=x3[:, :, :], in0=x2[:, :, :], in1=xa, op=MUL)
                nc.gpsimd.tensor_tensor(
                    out=omx3[:, :, :], in0=omx2[:, :, :], in1=omx[:, :, :], op=MUL)
                for (a, src, scl, other) in (
                    (0, omx3, None, None),
                    (1, xin, 3.0, omx2),
                    (2, x2, 3.0, omx),
                    (3, x3, None, None),
                ):
                    src_ap = _ap(src[:, :, :], 0, [[DIM, S], [1, NJ]])
                    out_ap = _ap(Bbig[:, :, :, :], a, [[4 * NJ, S], [4, NJ]])
                    if scl is None:
                        nc.vector.tensor_copy(out=out_ap, in_=src_ap)
                    else:
                        nc.vector.scalar_tensor_tensor(
                            out=out_ap, in0=src_ap, scalar=scl,
                            in1=_ap(other[:, :, :], 0, [[DIM, S], [1, NJ]]),
                            op0=MUL, op1=MUL)

            def emit_prologue_b0():
                """Fast-path mini A/B for group 0 (s in [0,4)), all on DVE."""
                pomx = singles.tile([P, NT, DIM], F32)
                pw = singles.tile([P, 4, NT, DIM], F32)  # x2, omx2, x3, omx3
                bb0 = singles.tile([P, NT, NJ, 4], F32)
                xa0 = _ap(xin[:, :, :], 0, [[DIM, NT], [1, DIM]])
                pa = [_ap(pw[:, :, :, :], q * NT * DIM, [[DIM, NT], [1, DIM]])
                      for q in range(4)]
                oa = _ap(pomx[:, :, :], 0, [[DIM, NT], [1, DIM]])
                nc.vector.tensor_scalar(
                    out=oa, in0=xa0, scalar1=-1.0, scalar2=1.0, op0=MUL, op1=ADD)
                nc.vector.tensor_tensor(out=pa[0], in0=xa0, in1=xa0, op=MUL)
                nc.vector.tensor_tensor(out=pa[1], in0=oa, in1=oa, op=MUL)
                nc.vector.tensor_tensor(out=pa[2], in0=pa[0], in1=xa0, op=MUL)
                nc.vector.tensor_tensor(out=pa[3], in0=pa[1], in1=oa, op=MUL)
                for (a, src, scl, other) in (
                    (0, pa[3], None, None),
                    (1, xa0, 3.0, pa[1]),
                    (2, pa[0], 3.0, oa),
                    (3, pa[2], None, None),
                ):
                    src_ap = bass.AP(tensor=src.tensor, offset=src.offset,
                                     ap=[src.ap[0], [DIM, NT], [1, NJ]])
                    out_ap = _ap(bb0[:, :, :, :], a, [[4 * NJ, NT], [4, NJ]])
                    if scl is None:
                        nc.vector.tensor_copy(out=out_ap, in_=src_ap)
                    else:
                        oth = bass.AP(tensor=other.tensor, offset=other.offset,
                                      ap=[other.ap[0], [DIM, NT], [1, NJ]])
                        nc.vector.scalar_tensor_tensor(
                            out=out_ap, in0=src_ap, scalar=scl, in1=oth,
                            op0=MUL, op1=MUL)
                return bb0

            # ---- vtab natural [P, S, 96] (f32r for PE transpose) ----
            vtab = bigs.tile([P, S, DIM, 16], BF16)
            t1f = bigs.tile([P, S, 4], F32)
            dq = bigs.tile([P, S], F32)
            tfsb = bigs.tile([M96, NG, NB], BF16)  # stashed tf96^T per group
            facn = bigs.tile([P, S, DIM], F32R)    # factors back in natural

            SD = S * DIM  # 384: ladders for all 6 dims at once
            px = powp.tile([P, 16, SD], F32, tag="px")
            pq = powp.tile([P, 16, SD], F32, tag="pq")

            def ladder_chunk(i):
                """Emit chunk i of the px (DVE) + pq (Pool) power ladders."""
                for (tbl, base, eng) in ((px, xin, nc.vector), (pq, omx, nc.gpsimd)):
                    t1 = tbl[:, :, :]
                    if i == 0:
                        eng.memset(tbl[:, 0, :], 1.0)
                        eng.tensor_copy(
                            out=tbl[:, 1, :], in_=_ap(base[:, :, :], 0, [[1, SD]]))
                        eng.tensor_tensor(
                            out=tbl[:, 2, :], in0=t1[:, 1, :], in1=t1[:, 1, :],
                            op=MUL)
                    elif i == 1:
                        eng.tensor_tensor(
                            out=_ap(t1, 3 * SD, [[1, 2 * SD]]),
                            in0=_ap(t1, SD, [[1, 2 * SD]]),
                            in1=_ap(t1, 2 * SD, [[0, 2], [1, SD]]), op=MUL)
                    elif i == 2:
                        eng.tensor_tensor(
                            out=_ap(t1, 5 * SD, [[1, 4 * SD]]),
                            in0=_ap(t1, SD, [[1, 4 * SD]]),
                            in1=_ap(t1, 4 * SD, [[0, 4], [1, SD]]), op=MUL)
                    else:
                        eng.tensor_tensor(
                            out=_ap(t1, 9 * SD, [[1, 7 * SD]]),
                            in0=_ap(t1, SD, [[1, 7 * SD]]),
                            in1=_ap(t1, 8 * SD, [[0, 7], [1, SD]]), op=MUL)

            def emit_vtab(j):
                # vtab[:, s, j, m] = px[m, s, j] * pq[15-m, s, j]
                eng = nc.gpsimd if j % 2 == 0 else nc.vector
                eng.tensor_tensor(
                    out=_ap(vtab[:, :, :, :], j * 16, [[1, 16], [DIM * 16, S]]),
                    in0=_ap(px[:, :, :], j, [[SD, 16], [DIM, S]]),
                    in1=_ap(pq[:, :, :], 15 * SD + j, [[-SD, 16], [DIM, S]]), op=MUL)

            # ---- stage D: cond -> XBAR transpose -> matmul -> stash ----
            def emit_group_mm(g, bb=None, gb=None):
                if bb is None:
                    bb = Bbig[:, :, :, :]
                    gb = g * NT * NJ * 4
                TS = NJ * 4
                k2g = condp.tile([P, NT, 16], F32, tag="k2")
                k3g = condp.tile([P, NT, 64], F32, tag="k3")
                q34g = condp.tile([P, NT, 16], F32, tag="q34")
                ea = nc.vector if g % 2 == 0 else nc.gpsimd
                eb = nc.gpsimd if g % 2 == 0 else nc.vector
                ea.tensor_tensor(
                    out=k2g[:, :, :],
                    in0=_ap(bb, gb + 0, [[TS, NT], [1, 4], [0, 4]]),
                    in1=_ap(bb, gb + 4, [[TS, NT], [0, 4], [1, 4]]), op=MUL)
                ea.tensor_tensor(
                    out=k3g[:, :, :],
                    in0=_ap(k2g[:, :, :], 0, [[16, NT], [1, 16], [0, 4]]),
                    in1=_ap(bb, gb + 8, [[TS, NT], [0, 16], [1, 4]]), op=MUL)
                eb.tensor_tensor(
                    out=q34g[:, :, :],
                    in0=_ap(bb, gb + 12, [[TS, NT], [1, 4], [0, 4]]),
                    in1=_ap(bb, gb + 16, [[TS, NT], [0, 4], [1, 4]]), op=MUL)
                ctb = ctbp.tile([P, KCH, NB], BF16, tag="ctb")
                for t in range(NT):
                    cnd = condp.tile([P, CDIM], BF16, tag=f"cond{t}")
                    idx = 4 * g + t
                    eng = nc.vector if (idx * 25) % 64 < 25 else nc.gpsimd
                    eng.tensor_tensor(
                        out=cnd[:, :],
                        in0=_ap(k3g[:, :, :], t * 64, [[1, 64], [0, 16]]),
                        in1=_ap(q34g[:, :, :], t * 16, [[0, 64], [1, 16]]), op=MUL)
                    if t > 0:
                        # XBAR: ctb[c%128, c//128, t*128+p] = cnd[p, c]
                        nc.sync.dma_start_transpose(
                            out=_ap(ctb[:, :, :], t * P, [[NB, KCH], [1, P]]),
                            in_=cnd[:, :])
                    else:
                        # PE transpose (8 blocks) + Act copy to offload XBAR
                        pct = ps_ct.tile([P, CDIM], BF16, tag="pct")
                        for k in range(KCH):
                            nc.tensor.matmul(
                                out=_ap(pct[:, :], k * P, [[1, P]]),
                                lhsT=cnd[:, k * P:(k + 1) * P],
                                rhs=idn[:, :],
                                is_transpose=True, start=True, stop=True,
                                skip_group_check=True)
                        nc.scalar.copy(
                            out=_ap(ctb[:, :, :], t * P, [[NB, KCH], [1, P]]),
                            in_=pct[:, :])
                tfps = ps_tf.tile([M96, NB], F32, tag="tf")
                for k in range(KCH):
                    nc.tensor.matmul(
                        out=tfps[:, :],
                        lhsT=wsb[:, k, :],
                        rhs=ctb[:, k, :],
                        start=(k == 0), stop=(k == KCH - 1))
                if g < NG - 2:
                    nc.scalar.copy(
                        out=_ap(tfsb[:, :, :], g * NB, [[1, NB]]),
                        in_=tfps[:, :])
                else:
                    nc.vector.tensor_copy(
                        out=_ap(tfsb[:, :, :], g * NB, [[1, NB]]),
                        in_=tfps[:, :])

            # ---- stage E: transposed combine per group ----
            def emit_group_combine(g):
                ppsi = ps_psi.tile([M96, NB], BF16, tag="ppsi")
                for t in range(NT):
                    nc.tensor.matmul(
                        out=_ap(ppsi[:, :], t * P, [[1, P]]),
                        lhsT=_ap(vtab[:, :, :, :], (4 * g + t) * M96, [[1, M96]]),
                        rhs=idn[:, :],
                        is_transpose=True, start=True, stop=True,
                        skip_group_check=True)
                psiT = eprodp.tile([M96, NB], BF16, tag="psiT")
                if g % 3 == 0:
                    nc.scalar.copy(out=psiT[:, :], in_=ppsi[:, :])
                else:
                    nc.vector.tensor_copy(out=psiT[:, :], in_=ppsi[:, :])
                eprod = eprodp.tile([M96, NB], BF16, tag="eprod")
                nc.gpsimd.tensor_tensor(
                    out=eprod[:, :],
                    in0=_ap(tfsb[:, :, :], g * NB, [[1, NB]]),
                    in1=psiT[:, :], op=MUL)
                fac = ps_fac.tile([DIM, NB], F32, tag="fac")
                nc.tensor.matmul(
                    out=fac[:, :], lhsT=e96[:, :], rhs=eprod[:, :],
                    start=True, stop=True)
                facsb = eprodp.tile([DIM, NB], F32R, tag="facsb")
                if g < NG - 3:
                    nc.scalar.copy(out=facsb[:, :], in_=fac[:, :])
                else:
                    nc.vector.tensor_copy(out=facsb[:, :], in_=fac[:, :])
                facT = ps_ft.tile([P, NT, DIM], F32R, tag="facT")
                for t in range(NT):
                    nc.tensor.matmul(
                        out=facT[:, t, :],
                        lhsT=_ap(facsb[:, :], t * P, [[1, P]]),
                        rhs=idnr[:DIM, :DIM],
                        is_transpose=True, start=True, stop=True,
                        skip_group_check=True)
                nc.scalar.copy(
                    out=_ap(facn[:, :, :], 4 * g * DIM, [[1, NT * DIM]]),
                    in_=facT[:, :, :])
                # incremental density product for this group's s-range
                eng3 = nc.vector if g % 2 == 0 else nc.gpsimd
                eng3.tensor_tensor(
                    out=_ap(t1f[:, :, :], 4 * g * 4, [[4, NT], [1, 3]]),
                    in0=_ap(facn[:, :, :], 4 * g * DIM, [[DIM, NT], [2, 3]]),
                    in1=_ap(facn[:, :, :], 4 * g * DIM + 1, [[DIM, NT], [2, 3]]),
                    op=MUL)
                eng3.tensor_tensor(
                    out=_ap(t1f[:, :, :], 4 * g * 4 + 3, [[4, NT]]),
                    in0=_ap(t1f[:, :, :], 4 * g * 4, [[4, NT]]),
                    in1=_ap(t1f[:, :, :], 4 * g * 4 + 1, [[4, NT]]), op=MUL)
                eng3.tensor_tensor(
                    out=_ap(dq[:, :], 4 * g, [[1, NT]]),
                    in0=_ap(t1f[:, :, :], 4 * g * 4 + 3, [[4, NT]]),
                    in1=_ap(t1f[:, :, :], 4 * g * 4 + 2, [[4, NT]]), op=MUL)
                if g == 7:
                    nc.sync.dma_start(out=dens_out[:, :32], in_=dq[:, :32])

            # ---- schedule: group-0 fast path, then full tables; ladders/vtab
            # interleaved into groups 1..6, combines trail mm by K groups ----
            K = 6
            emit_stages_ab()
            for g in range(0, NG):
                emit_group_mm(g)
                if g < 4:
                    ladder_chunk(g)
                elif g == 4:
                    emit_vtab(0), emit_vtab(1), emit_vtab(2)
                elif g == 5:
                    emit_vtab(3), emit_vtab(4), emit_vtab(5)
                if g >= K:
                    emit_group_combine(g - K)
            for g in range(NG - K, NG):
                emit_group_combine(g)

            # ---- final: second half of density already computed per group
            nc.sync.dma_start(out=dens_out[:, 32:], in_=dq[:, 32:])

    nc.finalize()
    return nc


def _softplus64(v):
    return np.logaddexp(0.0, v)


def _host_w(As):
    """W96 [1024, 96]: col i*16+m = kap_m * (ca_m - ca_{m-1}), ca_{-1}=0,
    ca_15=1 (binomial scaling + coeff finite-difference folded in)."""
    kap = 16.0 * np.array([math.comb(15, m) for m in range(16)], dtype=np.float64)
    cols = []
    for i in range(DIM):
        c = np.cumsum(_softplus64(As[i].astype(np.float64)), axis=1)
        ca = 2.0 * (1.0 / (1.0 + np.exp(-c)) - 0.5)
        rows = ca.shape[0]
        ext = np.concatenate(
            [np.zeros((rows, 1)), ca, np.ones((rows, 1))], axis=1)  # [r, 17]
        blk = kap * (ext[:, 1:] - ext[:, :-1])                      # [r, 16]
        cols.append(np.repeat(blk, 4 ** (5 - i), axis=0))
    return np.concatenate(cols, axis=1).astype(np.float32)


def kernel(**inputs):
    x = np.asarray(inputs["x"], dtype=np.float32)
    As = [np.asarray(inputs[f"A{i}"], dtype=np.float32) for i in range(DIM)]

    if "nc" not in _CACHE:
        _CACHE["nc"] = _build_nc()
    nc = _CACHE["nc"]

    w = _host_w(As)
    idn = np.eye(P, dtype=np.float32)
    e96 = np.zeros((M96, DIM), dtype=np.float32)
    for i in range(DIM):
        e96[i * 16:(i + 1) * 16, i] = 1.0

    in_maps = []
    for c in range(NCORES):
        xc = x[c * NC:(c + 1) * NC].reshape(P, S, DIM)
        in_maps.append({"xr": xc, "wmat": w, "ident": idn, "e96r": e96})

    res = run_bass_kernel_spmd(nc, in_maps, core_ids=list(range(NCORES)))
    outs = [r["dens"].reshape(NC) for r in res.results]
    return np.concatenate(outs, axis=0)


if __name__ == "__main__":
    rng = np.random.default_rng(0)
    ins = {"x": rng.uniform(0, 1, (N, DIM)).astype(np.float32)}
    for i in range(DIM):
        ins[f"A{i}"] = rng.uniform(0, 1, ((4 ** i), 15)).astype(np.float32)
    out = kernel(**ins)
    print(out.shape, out[:4])


# revision 105
# speedup vs baseline: 1.0240x; 1.0049x over previous
"""Bernstein flow density kernel for 8x TRN2 NeuronCores.

Math (per sample n):
  density(n) = prod_i [ phi_i[n,15] + sum_m tf_i[n,m] * psi_i[n,m] ]
  tf_i = cond_i @ c_alpha_i,  cond_i = B_0 (x) ... (x) B_{i-1}  (row-wise Kron)
Bernstein bases sum to 1, so cond_i is a marginal of cond_5 [N,1024] and all
six matmuls merge into ONE with a 96-wide W.  Both the binomial scaling and
the coefficient finite-difference fold into W host-side:
  factor_i = sum_{m=0..15} (ca_m - ca_{m-1}) * kap_m * x^m (1-x)^(15-m)
  W96[c, i*16+m] = kap_m * (ca_i[pre,m] - ca_i[pre,m-1]),  ca_{-1}=0, ca_15=1
  tf96 = cond_5 @ W96;  vtab[n, i,m] = x_i^m (1-x_i)^(15-m)
  factor_i = sum_m tf96 * vtab  ->  density = prod_i factor_i

Per core (8192 samples, p-major: local n = p*64 + s), per group g of 512
samples (s = 4g..4g+3):
  1. DVE/Pool build deg-3 tables, k3/q34, then cond tiles [128,1024] in bf16
  2. XBAR DMA transpose (SBUF->SBUF, 2-byte) -> ctb [128(c%128), 8, 512] bf16
  3. PE: 8 accumulating bf16 matmuls vs W96 chunks -> tf96^T [96,512] psum
     (moving dim 512 -> 1 cycle/row), Act stashes to SBUF
  4. PE transposes vtab natural [128,96] -> v^T [96,512] (bf16 identity),
     eprod = tf^T (*) v^T, E-matmul reduces (i,m)->i -> factors [6,512],
     tiny back-transpose -> natural [128, s, 6]; final 3-op product -> dens
"""

import math
import sys

import numpy as np

sys.path.insert(0, "/opt/trn_rl_repo")

import concourse.bacc as bacc  # noqa: E402
import concourse.bass as bass  # noqa: E402
import concourse.tile as tile  # noqa: E402
from concourse import mybir  # noqa: E402
from concourse.bass_utils import run_bass_kernel_spmd  # noqa: E402

N = 65536
DIM = 6
NCORES = 8
NC = N // NCORES          # 8192 samples per core
P = 128
S = NC // P               # 64 samples per partition
NT = 4                    # s-tiles per group
NG = S // NT              # 16 groups (512 samples each)
NB = NT * P               # 512 samples per group
CDIM = 1024               # cond_5 width
KCH = CDIM // P           # 8 contraction chunks
M96 = 96                  # 6 dims * 16 coeffs (m=15 carries phi)

F32 = mybir.dt.float32
F32R = mybir.dt.float32r
BF16 = mybir.dt.bfloat16
MUL = mybir.AluOpType.mult
ADD = mybir.AluOpType.add
SUB = mybir.AluOpType.subtract
AF = mybir.ActivationFunctionType

_CACHE = {}


def _ap(a, off_elems, dims):
    """AP over slice a with replaced free dims; dims = [[step,count],...]."""
    return bass.AP(tensor=a.tensor, offset=a.offset + off_elems, ap=[a.ap[0]] + dims)


def _build_nc():
    nc = bacc.Bacc(target_bir_lowering=False, trn_type="TRN2")

    xr = nc.dram_tensor("xr", [P, S, DIM], F32, kind="ExternalInput")
    wmat = nc.dram_tensor("wmat", [CDIM, M96], F32, kind="ExternalInput")
    ident = nc.dram_tensor("ident", [P, P], F32, kind="ExternalInput")
    e96r = nc.dram_tensor("e96r", [M96, DIM], F32, kind="ExternalInput")
    dens_out = nc.dram_tensor("dens", [P, S], F32, kind="ExternalOutput")

    with tile.TileContext(nc) as tc:
        with (
            tc.tile_pool(name="singles", bufs=1) as singles,
            tc.tile_pool(name="bigs", bufs=1) as bigs,
            tc.tile_pool(name="cond", bufs=6) as condp,
            tc.tile_pool(name="ctb", bufs=3) as ctbp,
            tc.tile_pool(name="pows", bufs=1) as powp,
            tc.tile_pool(name="eprodp", bufs=2) as eprodp,
            tc.tile_pool(name="ps_tf", bufs=2, space="PSUM") as ps_tf,
            tc.tile_pool(name="ps_psi", bufs=2, space="PSUM") as ps_psi,
            tc.tile_pool(name="ps_ct", bufs=1, space="PSUM") as ps_ct,
            tc.tile_pool(name="ps_fac", bufs=2, space="PSUM") as ps_fac,
            tc.tile_pool(name="ps_ft", bufs=1, space="PSUM") as ps_ft,
        ):
            # ---- constants / inputs ----
            xin = singles.tile([P, S, DIM], F32)
            nc.sync.dma_start(out=xin[:, :, :], in_=xr[:, :, :])
            idf = singles.tile([P, P], F32)
            nc.sync.dma_start(out=idf[:, :], in_=ident[:, :])
            wf = singles.tile([P, KCH, M96], F32)
            nc.sync.dma_start(
                out=wf[:, :, :],
                in_=bass.AP(tensor=wmat[:, :].tensor, offset=0,
                            ap=[[M96, P], [P * M96, KCH], [1, M96]]),
            )
            # bf16 casts: W chunks + identity (rhs of transposes -> 1 cyc/row)
            # idnr: f32r identity for f32r-lhsT transposes (walrus requires
            # fp32/f32r operands to match dtypes exactly)
            wsb = singles.tile([P, KCH, M96], BF16)
            nc.vector.tensor_copy(out=wsb[:, :, :], in_=wf[:, :, :])
            idn = singles.tile([P, P], BF16)
            nc.vector.tensor_copy(out=idn[:, :], in_=idf[:, :])
            idnr = singles.tile([P, P], F32R)
            nc.vector.tensor_copy(out=idnr[:, :], in_=idf[:, :])
            # E96 [96, 6] f32r: block one-hot reducing (i,m)->i
            e96f = singles.tile([M96, DIM], F32)
            nc.sync.dma_start(out=e96f[:, :], in_=e96r[:, :])
            e96 = singles.tile([M96, DIM], BF16)
            nc.vector.tensor_copy(out=e96[:, :], in_=e96f[:, :])

            xa = xin[:, :, :]
            NJ = 5

            # ---- stage A/B: powers + deg-3 tables Bbig[p, s, j, a] ----
            omx = singles.tile([P, S, DIM], F32)
            x2 = singles.tile([P, S, DIM], F32)
            x3 = singles.tile([P, S, DIM], F32)
            omx2 = singles.tile([P, S, DIM], F32)
            omx3 = singles.tile([P, S, DIM], F32)
            Bbig = singles.tile([P, S, NJ, 4], F32)

            def emit_stages_ab():
                nc.vector.tensor_scalar(
                    out=omx[:, :, :], in0=xa, scalar1=-1.0, scalar2=1.0,
                    op0=MUL, op1=ADD)
                nc.scalar.square(out=x2[:, :, :], in_=xa)
                nc.scalar.square(out=omx2[:, :, :], in_=omx[:, :, :])
                nc.vector.tensor_tensor(
                    out=x3[:, :, :], in0=x2[:, :, :], in1=xa, op=MUL)
                nc.gpsimd.tensor_tensor(
                    out=omx3[:, :, :], in0=omx2[:, :, :], in1=omx[:, :, :], op=MUL)
                for (a, src, scl, other) in (
                    (0, omx3, None, None),
                    (1, xin, 3.0, omx2),
                    (2, x2, 3.0, omx),
                    (3, x3, None, None),
                ):
                    src_ap = _ap(src[:, :, :], 0, [[DIM, S], [1, NJ]])
                    out_ap = _ap(Bbig[:, :, :, :], a, [[4 * NJ, S], [4, NJ]])
                    if scl is None:
                        nc.vector.tensor_copy(out=out_ap, in_=src_ap)
                    else:
                        nc.vector.scalar_tensor_tensor(
                            out=out_ap, in0=src_ap, scalar=scl,
                            in1=_ap(other[:, :, :], 0, [[DIM, S], [1, NJ]]),
                            op0=MUL, op1=MUL)

            def emit_prologue_b0():
                """Fast-path mini A/B for group 0 (s in [0,4)), all on DVE."""
                pomx = singles.tile([P, NT, DIM], F32)
                pw = singles.tile([P, 4, NT, DIM], F32)  # x2, omx2, x3, omx3
                bb0 = singles.tile([P, NT, NJ, 4], F32)
                xa0 = _ap(xin[:, :, :], 0, [[DIM, NT], [1, DIM]])
                pa = [_ap(pw[:, :, :, :], q * NT * DIM, [[DIM, NT], [1, DIM]])
                      for q in range(4)]
                oa = _ap(pomx[:, :, :], 0, [[DIM, NT], [1, DIM]])
                nc.vector.tensor_scalar(
                    out=oa, in0=xa0, scalar1=-1.0, scalar2=1.0, op0=MUL, op1=ADD)
                nc.vector.tensor_tensor(out=pa[0], in0=xa0, in1=xa0, op=MUL)
                nc.vector.tensor_tensor(out=pa[1], in0=oa, in1=oa, op=MUL)
                nc.vector.tensor_tensor(out=pa[2], in0=pa[0], in1=xa0, op=MUL)
                nc.vector.tensor_tensor(out=pa[3], in0=pa[1], in1=oa, op=MUL)
                for (a, src, scl, other) in (
                    (0, pa[3], None, None),
                    (1, xa0, 3.0, pa[1]),
                    (2, pa[0], 3.0, oa),
                    (3, pa[2], None, None),
                ):
                    src_ap = bass.AP(tensor=src.tensor, offset=src.offset,
                                     ap=[src.ap[0], [DIM, NT], [1, NJ]])
                    out_ap = _ap(bb0[:, :, :, :], a, [[4 * NJ, NT], [4, NJ]])
                    if scl is None:
                        nc.vector.tensor_copy(out=out_ap, in_=src_ap)
                    else:
                        oth = bass.AP(tensor=other.tensor, offset=other.offset,
                                      ap=[other.ap[0], [DIM, NT], [1, NJ]])
                        nc.vector.scalar_tensor_tensor(
                            out=out_ap, in0=src_ap, scalar=scl, in1=oth,
                            op0=MUL, op1=MUL)
                return bb0

            # ---- vtab natural [P, S, 96] (f32r for PE transpose) ----
            vtab = bigs.tile([P, S, DIM, 16], BF16)
            t1f = bigs.tile([P, S, 4], F32)
            dq = bigs.tile([P, S], F32)
            tfsb = bigs.tile([M96, NG, NB], BF16)  # stashed tf96^T per group
            facn = bigs.tile([P, S, DIM], F32R)    # factors back in natural

            SD = S * DIM  # 384: ladders for all 6 dims at once
            px = powp.tile([P, 16, SD], F32, tag="px")
            pq = powp.tile([P, 16, SD], F32, tag="pq")

            def ladder_chunk(i):
                """Emit chunk i of the px (DVE) + pq (Pool) power ladders."""
                for (tbl, base, eng) in ((px, xin, nc.vector), (pq, omx, nc.gpsimd)):
                    t1 = tbl[:, :, :]
                    if i == 0:
                        eng.memset(tbl[:, 0, :], 1.0)
                        eng.tensor_copy(
                            out=tbl[:, 1, :], in_=_ap(base[:, :, :], 0, [[1, SD]]))
                        eng.tensor_tensor(
                            out=tbl[:, 2, :], in0=t1[:, 1, :], in1=t1[:, 1, :],
                            op=MUL)
                    elif i == 1:
                        eng.tensor_tensor(
                            out=_ap(t1, 3 * SD, [[1, 2 * SD]]),
                            in0=_ap(t1, SD, [[1, 2 * SD]]),
                            in1=_ap(t1, 2 * SD, [[0, 2], [1, SD]]), op=MUL)
                    elif i == 2:
                        eng.tensor_tensor(
                            out=_ap(t1, 5 * SD, [[1, 4 * SD]]),
                            in0=_ap(t1, SD, [[1, 4 * SD]]),
                            in1=_ap(t1, 4 * SD, [[0, 4], [1, SD]]), op=MUL)
                    else:
                        eng.tensor_tensor(
                            out=_ap(t1, 9 * SD, [[1, 7 * SD]]),
                            in0=_ap(t1, SD, [[1, 7 * SD]]),
                            in1=_ap(t1, 8 * SD, [[0, 7], [1, SD]]), op=MUL)

            def emit_vtab(j):
                # vtab[:, s, j, m] = px[m, s, j] * pq[15-m, s, j]
                eng = nc.gpsimd if j % 2 == 0 or j == 5 else nc.vector
                eng.tensor_tensor(
                    out=_ap(vtab[:, :, :, :], j * 16, [[1, 16], [DIM * 16, S]]),
                    in0=_ap(px[:, :, :], j, [[SD, 16], [DIM, S]]),
                    in1=_ap(pq[:, :, :], 15 * SD + j, [[-SD, 16], [DIM, S]]), op=MUL)

            # ---- stage D: cond -> XBAR transpose -> matmul -> stash ----
            def emit_group_mm(g, bb=None, gb=None):
                if bb is None:
                    bb = Bbig[:, :, :, :]
                    gb = g * NT * NJ * 4
                TS = NJ * 4
                k2g = condp.tile([P, NT, 16], F32, tag="k2")
                k3g = condp.tile([P, NT, 64], F32, tag="k3")
                q34g = condp.tile([P, NT, 16], F32, tag="q34")
                ea = nc.vector if g % 2 == 0 else nc.gpsimd
                eb = nc.gpsimd if g % 2 == 0 else nc.vector
                ea.tensor_tensor(
                    out=k2g[:, :, :],
                    in0=_ap(bb, gb + 0, [[TS, NT], [1, 4], [0, 4]]),
                    in1=_ap(bb, gb + 4, [[TS, NT], [0, 4], [1, 4]]), op=MUL)
                ea.tensor_tensor(
                    out=k3g[:, :, :],
                    in0=_ap(k2g[:, :, :], 0, [[16, NT], [1, 16], [0, 4]]),
                    in1=_ap(bb, gb + 8, [[TS, NT], [0, 16], [1, 4]]), op=MUL)
                eb.tensor_tensor(
                    out=q34g[:, :, :],
                    in0=_ap(bb, gb + 12, [[TS, NT], [1, 4], [0, 4]]),
                    in1=_ap(bb, gb + 16, [[TS, NT], [0, 4], [1, 4]]), op=MUL)
                ctb = ctbp.tile([P, KCH, NB], BF16, tag="ctb")
                for t in range(NT):
                    cnd = condp.tile([P, CDIM], BF16, tag=f"cond{t}")
                    idx = 4 * g + t
                    eng = nc.vector if (idx * 25) % 64 < 25 else nc.gpsimd
                    eng.tensor_tensor(
                        out=cnd[:, :],
                        in0=_ap(k3g[:, :, :], t * 64, [[1, 64], [0, 16]]),
                        in1=_ap(q34g[:, :, :], t * 16, [[0, 64], [1, 16]]), op=MUL)
                    if t > 0:
                        # XBAR: ctb[c%128, c//128, t*128+p] = cnd[p, c]
                        nc.sync.dma_start_transpose(
                            out=_ap(ctb[:, :, :], t * P, [[NB, KCH], [1, P]]),
                            in_=cnd[:, :])
                    else:
                        # PE transpose (8 blocks) + Act copy to offload XBAR
                        pct = ps_ct.tile([P, CDIM], BF16, tag="pct")
                        for k in range(KCH):
                            nc.tensor.matmul(
                                out=_ap(pct[:, :], k * P, [[1, P]]),
                                lhsT=cnd[:, k * P:(k + 1) * P],
                                rhs=idn[:, :],
                                is_transpose=True, start=True, stop=True,
                                skip_group_check=True)
                        nc.scalar.copy(
                            out=_ap(ctb[:, :, :], t * P, [[NB, KCH], [1, P]]),
                            in_=pct[:, :])
                tfps = ps_tf.tile([M96, NB], F32, tag="tf")
                for k in range(KCH):
                    nc.tensor.matmul(
                        out=tfps[:, :],
                        lhsT=wsb[:, k, :],
                        rhs=ctb[:, k, :],
                        start=(k == 0), stop=(k == KCH - 1))
                if g < NG - 2:
                    nc.scalar.copy(
                        out=_ap(tfsb[:, :, :], g * NB, [[1, NB]]),
                        in_=tfps[:, :])
                else:
                    nc.vector.tensor_copy(
                        out=_ap(tfsb[:, :, :], g * NB, [[1, NB]]),
                        in_=tfps[:, :])

            # ---- stage E: transposed combine per group ----
            def emit_group_combine(g):
                ppsi = ps_psi.tile([M96, NB], BF16, tag="ppsi")
                for t in range(NT):
                    nc.tensor.matmul(
                        out=_ap(ppsi[:, :], t * P, [[1, P]]),
                        lhsT=_ap(vtab[:, :, :, :], (4 * g + t) * M96, [[1, M96]]),
                        rhs=idn[:, :],
                        is_transpose=True, start=True, stop=True,
                        skip_group_check=True)
                psiT = eprodp.tile([M96, NB], BF16, tag="psiT")
                if g % 3 == 0:
                    nc.scalar.copy(out=psiT[:, :], in_=ppsi[:, :])
                else:
                    nc.vector.tensor_copy(out=psiT[:, :], in_=ppsi[:, :])
                eprod = eprodp.tile([M96, NB], BF16, tag="eprod")
                nc.gpsimd.tensor_tensor(
                    out=eprod[:, :],
                    in0=_ap(tfsb[:, :, :], g * NB, [[1, NB]]),
                    in1=psiT[:, :], op=MUL)
                fac = ps_fac.tile([DIM, NB], F32, tag="fac")
                nc.tensor.matmul(
                    out=fac[:, :], lhsT=e96[:, :], rhs=eprod[:, :],
                    start=True, stop=True)
                facsb = eprodp.tile([DIM, NB], F32R, tag="facsb")
                if g < NG - 3:
                    nc.scalar.copy(out=facsb[:, :], in_=fac[:, :])
                else:
                    nc.vector.tensor_copy(out=facsb[:, :], in_=fac[:, :])
                facT = ps_ft.tile([P, NT, DIM], F32R, tag="facT")
                for t in range(NT):
                    nc.tensor.matmul(
                        out=facT[:, t, :],
                        lhsT=_ap(facsb[:, :], t * P, [[1, P]]),
                        rhs=idnr[:DIM, :DIM],
                        is_transpose=True, start=True, stop=True,
                        skip_group_check=True)
                nc.scalar.copy(
                    out=_ap(facn[:, :, :], 4 * g * DIM, [[1, NT * DIM]]),
                    in_=facT[:, :, :])
                # incremental density product for this group's s-range
                eng3 = nc.vector if g % 2 == 0 else nc.gpsimd
                eng3.tensor_tensor(
                    out=_ap(t1f[:, :, :], 4 * g * 4, [[4, NT], [1, 3]]),
                    in0=_ap(facn[:, :, :], 4 * g * DIM, [[DIM, NT], [2, 3]]),
                    in1=_ap(facn[:, :, :], 4 * g * DIM + 1, [[DIM, NT], [2, 3]]),
                    op=MUL)
                eng3.tensor_tensor(
                    out=_ap(t1f[:, :, :], 4 * g * 4 + 3, [[4, NT]]),
                    in0=_ap(t1f[:, :, :], 4 * g * 4, [[4, NT]]),
                    in1=_ap(t1f[:, :, :], 4 * g * 4 + 1, [[4, NT]]), op=MUL)
                eng3.tensor_tensor(
                    out=_ap(dq[:, :], 4 * g, [[1, NT]]),
                    in0=_ap(t1f[:, :, :], 4 * g * 4 + 3, [[4, NT]]),
                    in1=_ap(t1f[:, :, :], 4 * g * 4 + 2, [[4, NT]]), op=MUL)
                if g == 7:
                    nc.sync.dma_start(out=dens_out[:, :32], in_=dq[:, :32])

            # ---- schedule: group-0 fast path, then full tables; ladders/vtab
            # interleaved into groups 1..6, combines trail mm by K groups ----
            K = 6
            emit_stages_ab()
            for g in range(0, NG):
                emit_group_mm(g)
                if g < 4:
                    ladder_chunk(g)
                elif g == 4:
                    emit_vtab(0), emit_vtab(1), emit_vtab(2)
                elif g == 5:
                    emit_vtab(3), emit_vtab(4), emit_vtab(5)
                if g >= K:
                    emit_group_combine(g - K)
            for g in range(NG - K, NG):
                emit_group_combine(g)

            # ---- final: second half of density already computed per group
            nc.sync.dma_start(out=dens_out[:, 32:], in_=dq[:, 32:])

    nc.finalize()
    return nc


def _softplus64(v):
    return np.logaddexp(0.0, v)


def _host_w(As):
    """W96 [1024, 96]: col i*16+m = kap_m * (ca_m - ca_{m-1}), ca_{-1}=0,
    ca_15=1 (binomial scaling + coeff finite-difference folded in)."""
    kap = 16.0 * np.array([math.comb(15, m) for m in range(16)], dtype=np.float64)
    cols = []
    for i in range(DIM):
        c = np.cumsum(_softplus64(As[i].astype(np.float64)), axis=1)
        ca = 2.0 * (1.0 / (1.0 + np.exp(-c)) - 0.5)
        rows = ca.shape[0]
        ext = np.concatenate(
            [np.zeros((rows, 1)), ca, np.ones((rows, 1))], axis=1)  # [r, 17]
        blk = kap * (ext[:, 1:] - ext[:, :-1])                      # [r, 16]
        cols.append(np.repeat(blk, 4 ** (5 - i), axis=0))
    return np.concatenate(cols, axis=1).astype(np.float32)


def kernel(**inputs):
    x = np.asarray(inputs["x"], dtype=np.float32)
    As = [np.asarray(inputs[f"A{i}"], dtype=np.float32) for i in range(DIM)]

    if "nc" not in _CACHE:
        _CACHE["nc"] = _build_nc()
    nc = _CACHE["nc"]

    w = _host_w(As)
    idn = np.eye(P, dtype=np.float32)
    e96 = np.zeros((M96, DIM), dtype=np.float32)
    for i in range(DIM):
        e96[i * 16:(i + 1) * 16, i] = 1.0

    in_maps = []
    for c in range(NCORES):
        xc = x[c * NC:(c + 1) * NC].reshape(P, S, DIM)
        in_maps.append({"xr": xc, "wmat": w, "ident": idn, "e96r": e96})

    res = run_bass_kernel_spmd(nc, in_maps, core_ids=list(range(NCORES)))
    outs = [r["dens"].reshape(NC) for r in res.results]
    return np.concatenate(outs, axis=0)


if __name__ == "__main__":
    rng = np.random.default_rng(0)
    ins = {"x": rng.uniform(0, 1, (N, DIM)).astype(np.float32)}
    for i in range(DIM):
        ins[f"A{i}"] = rng.uniform(0, 1, ((4 ** i), 15)).astype(np.float32)
    out = kernel(**ins)
    print(out.shape, out[:4])


# revision 109
# speedup vs baseline: 1.0243x; 1.0003x over previous
"""Bernstein flow density kernel for 8x TRN2 NeuronCores.

Math (per sample n):
  density(n) = prod_i [ phi_i[n,15] + sum_m tf_i[n,m] * psi_i[n,m] ]
  tf_i = cond_i @ c_alpha_i,  cond_i = B_0 (x) ... (x) B_{i-1}  (row-wise Kron)
Bernstein bases sum to 1, so cond_i is a marginal of cond_5 [N,1024] and all
six matmuls merge into ONE with a 96-wide W.  Both the binomial scaling and
the coefficient finite-difference fold into W host-side:
  factor_i = sum_{m=0..15} (ca_m - ca_{m-1}) * kap_m * x^m (1-x)^(15-m)
  W96[c, i*16+m] = kap_m * (ca_i[pre,m] - ca_i[pre,m-1]),  ca_{-1}=0, ca_15=1
  tf96 = cond_5 @ W96;  vtab[n, i,m] = x_i^m (1-x_i)^(15-m)
  factor_i = sum_m tf96 * vtab  ->  density = prod_i factor_i

Per core (8192 samples, p-major: local n = p*64 + s), per group g of 512
samples (s = 4g..4g+3):
  1. DVE/Pool build deg-3 tables, k3/q34, then cond tiles [128,1024] in bf16
  2. XBAR DMA transpose (SBUF->SBUF, 2-byte) -> ctb [128(c%128), 8, 512] bf16
  3. PE: 8 accumulating bf16 matmuls vs W96 chunks -> tf96^T [96,512] psum
     (moving dim 512 -> 1 cycle/row), Act stashes to SBUF
  4. PE transposes vtab natural [128,96] -> v^T [96,512] (bf16 identity),
     eprod = tf^T (*) v^T, E-matmul reduces (i,m)->i -> factors [6,512],
     tiny back-transpose -> natural [128, s, 6]; final 3-op product -> dens
"""

import math
import sys

import numpy as np

sys.path.insert(0, "/opt/trn_rl_repo")

import concourse.bacc as bacc  # noqa: E402
import concourse.bass as bass  # noqa: E402
import concourse.tile as tile  # noqa: E402
from concourse import mybir  # noqa: E402
from concourse.bass_utils import run_bass_kernel_spmd  # noqa: E402

N = 65536
DIM = 6
NCORES = 8
NC = N // NCORES          # 8192 samples per core
P = 128
S = NC // P               # 64 samples per partition
NT = 4                    # s-tiles per group
NG = S // NT              # 16 groups (512 samples each)
NB = NT * P               # 512 samples per group
CDIM = 1024               # cond_5 width
KCH = CDIM // P           # 8 contraction chunks
M96 = 96                  # 6 dims * 16 coeffs (m=15 carries phi)

F32 = mybir.dt.float32
F32R = mybir.dt.float32r
BF16 = mybir.dt.bfloat16
MUL = mybir.AluOpType.mult
ADD = mybir.AluOpType.add
SUB = mybir.AluOpType.subtract
AF = mybir.ActivationFunctionType

_CACHE = {}


def _ap(a, off_elems, dims):
    """AP over slice a with replaced free dims; dims = [[step,count],...]."""
    return bass.AP(tensor=a.tensor, offset=a.offset + off_elems, ap=[a.ap[0]] + dims)


def _build_nc():
    nc = bacc.Bacc(target_bir_lowering=False, trn_type="TRN2")

    xr = nc.dram_tensor("xr", [P, S, DIM], F32, kind="ExternalInput")
    wmat = nc.dram_tensor("wmat", [CDIM, M96], F32, kind="ExternalInput")
    ident = nc.dram_tensor("ident", [P, P], F32, kind="ExternalInput")
    e96r = nc.dram_tensor("e96r", [M96, DIM], F32, kind="ExternalInput")
    dens_out = nc.dram_tensor("dens", [P, S], F32, kind="ExternalOutput")

    with tile.TileContext(nc) as tc:
        with (
            tc.tile_pool(name="singles", bufs=1) as singles,
            tc.tile_pool(name="bigs", bufs=1) as bigs,
            tc.tile_pool(name="cond", bufs=6) as condp,
            tc.tile_pool(name="ctb", bufs=3) as ctbp,
            tc.tile_pool(name="pows", bufs=1) as powp,
            tc.tile_pool(name="eprodp", bufs=2) as eprodp,
            tc.tile_pool(name="ps_tf", bufs=2, space="PSUM") as ps_tf,
            tc.tile_pool(name="ps_psi", bufs=2, space="PSUM") as ps_psi,
            tc.tile_pool(name="ps_ct", bufs=1, space="PSUM") as ps_ct,
            tc.tile_pool(name="ps_fac", bufs=2, space="PSUM") as ps_fac,
            tc.tile_pool(name="ps_ft", bufs=1, space="PSUM") as ps_ft,
        ):
            # ---- constants / inputs ----
            xin = singles.tile([P, S, DIM], F32)
            nc.sync.dma_start(out=xin[:, :, :], in_=xr[:, :, :])
            idf = singles.tile([P, P], F32)
            nc.sync.dma_start(out=idf[:, :], in_=ident[:, :])
            wf = singles.tile([P, KCH, M96], F32)
            nc.sync.dma_start(
                out=wf[:, :, :],
                in_=bass.AP(tensor=wmat[:, :].tensor, offset=0,
                            ap=[[M96, P], [P * M96, KCH], [1, M96]]),
            )
            # bf16 casts: W chunks + identity (rhs of transposes -> 1 cyc/row)
            # idnr: f32r identity for f32r-lhsT transposes (walrus requires
            # fp32/f32r operands to match dtypes exactly)
            wsb = singles.tile([P, KCH, M96], BF16)
            nc.vector.tensor_copy(out=wsb[:, :, :], in_=wf[:, :, :])
            idn = singles.tile([P, P], BF16)
            nc.vector.tensor_copy(out=idn[:, :], in_=idf[:, :])
            idnr = singles.tile([P, P], F32R)
            nc.vector.tensor_copy(out=idnr[:, :], in_=idf[:, :])
            # E96 [96, 6] f32r: block one-hot reducing (i,m)->i
            e96f = singles.tile([M96, DIM], F32)
            nc.sync.dma_start(out=e96f[:, :], in_=e96r[:, :])
            e96 = singles.tile([M96, DIM], BF16)
            nc.vector.tensor_copy(out=e96[:, :], in_=e96f[:, :])

            xa = xin[:, :, :]
            NJ = 5

            # ---- stage A/B: powers + deg-3 tables Bbig[p, s, j, a] ----
            omx = singles.tile([P, S, DIM], F32)
            x2 = singles.tile([P, S, DIM], F32)
            x3 = singles.tile([P, S, DIM], F32)
            omx2 = singles.tile([P, S, DIM], F32)
            omx3 = singles.tile([P, S, DIM], F32)
            Bbig = singles.tile([P, S, NJ, 4], F32)

            def emit_stages_ab():
                nc.vector.tensor_scalar(
                    out=omx[:, :, :], in0=xa, scalar1=-1.0, scalar2=1.0,
                    op0=MUL, op1=ADD)
                nc.scalar.square(out=x2[:, :, :], in_=xa)
                nc.scalar.square(out=omx2[:, :, :], in_=omx[:, :, :])
                nc.vector.tensor_tensor(
                    out=x3[:, :, :], in0=x2[:, :, :], in1=xa, op=MUL)
                nc.gpsimd.tensor_tensor(
                    out=omx3[:, :, :], in0=omx2[:, :, :], in1=omx[:, :, :], op=MUL)
                for (a, src, scl, other) in (
                    (0, omx3, None, None),
                    (1, xin, 3.0, omx2),
                    (2, x2, 3.0, omx),
                    (3, x3, None, None),
                ):
                    src_ap = _ap(src[:, :, :], 0, [[DIM, S], [1, NJ]])
                    out_ap = _ap(Bbig[:, :, :, :], a, [[4 * NJ, S], [4, NJ]])
                    if scl is None:
                        nc.vector.tensor_copy(out=out_ap, in_=src_ap)
                    else:
                        nc.vector.scalar_tensor_tensor(
                            out=out_ap, in0=src_ap, scalar=scl,
                            in1=_ap(other[:, :, :], 0, [[DIM, S], [1, NJ]]),
                            op0=MUL, op1=MUL)

            def emit_prologue_b0():
                """Fast-path mini A/B for group 0 (s in [0,4)), all on DVE."""
                pomx = singles.tile([P, NT, DIM], F32)
                pw = singles.tile([P, 4, NT, DIM], F32)  # x2, omx2, x3, omx3
                bb0 = singles.tile([P, NT, NJ, 4], F32)
                xa0 = _ap(xin[:, :, :], 0, [[DIM, NT], [1, DIM]])
                pa = [_ap(pw[:, :, :, :], q * NT * DIM, [[DIM, NT], [1, DIM]])
                      for q in range(4)]
                oa = _ap(pomx[:, :, :], 0, [[DIM, NT], [1, DIM]])
                nc.vector.tensor_scalar(
                    out=oa, in0=xa0, scalar1=-1.0, scalar2=1.0, op0=MUL, op1=ADD)
                nc.vector.tensor_tensor(out=pa[0], in0=xa0, in1=xa0, op=MUL)
                nc.vector.tensor_tensor(out=pa[1], in0=oa, in1=oa, op=MUL)
                nc.vector.tensor_tensor(out=pa[2], in0=pa[0], in1=xa0, op=MUL)
                nc.vector.tensor_tensor(out=pa[3], in0=pa[1], in1=oa, op=MUL)
                for (a, src, scl, other) in (
                    (0, pa[3], None, None),
                    (1, xa0, 3.0, pa[1]),
                    (2, pa[0], 3.0, oa),
                    (3, pa[2], None, None),
                ):
                    src_ap = bass.AP(tensor=src.tensor, offset=src.offset,
                                     ap=[src.ap[0], [DIM, NT], [1, NJ]])
                    out_ap = _ap(bb0[:, :, :, :], a, [[4 * NJ, NT], [4, NJ]])
                    if scl is None:
                        nc.vector.tensor_copy(out=out_ap, in_=src_ap)
                    else:
                        oth = bass.AP(tensor=other.tensor, offset=other.offset,
                                      ap=[other.ap[0], [DIM, NT], [1, NJ]])
                        nc.vector.scalar_tensor_tensor(
                            out=out_ap, in0=src_ap, scalar=scl, in1=oth,
                            op0=MUL, op1=MUL)
                return bb0

            # ---- vtab natural [P, S, 96] (f32r for PE transpose) ----
            vtab = bigs.tile([P, S, DIM, 16], BF16)
            t1f = bigs.tile([P, S, 4], F32)
            dq = bigs.tile([P, S], F32)
            tfsb = bigs.tile([M96, NG, NB], BF16)  # stashed tf96^T per group
            facn = bigs.tile([P, S, DIM], F32R)    # factors back in natural

            SD = S * DIM  # 384: ladders for all 6 dims at once
            px = powp.tile([P, 16, SD], F32, tag="px")
            pq = powp.tile([P, 16, SD], F32, tag="pq")

            def ladder_chunk(i):
                """Emit chunk i of the px (DVE) + pq (Pool) power ladders."""
                for (tbl, base, eng) in ((px, xin, nc.vector), (pq, omx, nc.gpsimd)):
                    t1 = tbl[:, :, :]
                    if i == 0:
                        eng.memset(tbl[:, 0, :], 1.0)
                        eng.tensor_copy(
                            out=tbl[:, 1, :], in_=_ap(base[:, :, :], 0, [[1, SD]]))
                        eng.tensor_tensor(
                            out=tbl[:, 2, :], in0=t1[:, 1, :], in1=t1[:, 1, :],
                            op=MUL)
                    elif i == 1:
                        eng.tensor_tensor(
                            out=_ap(t1, 3 * SD, [[1, 2 * SD]]),
                            in0=_ap(t1, SD, [[1, 2 * SD]]),
                            in1=_ap(t1, 2 * SD, [[0, 2], [1, SD]]), op=MUL)
                    elif i == 2:
                        eng.tensor_tensor(
                            out=_ap(t1, 5 * SD, [[1, 4 * SD]]),
                            in0=_ap(t1, SD, [[1, 4 * SD]]),
                            in1=_ap(t1, 4 * SD, [[0, 4], [1, SD]]), op=MUL)
                    else:
                        eng.tensor_tensor(
                            out=_ap(t1, 9 * SD, [[1, 7 * SD]]),
                            in0=_ap(t1, SD, [[1, 7 * SD]]),
                            in1=_ap(t1, 8 * SD, [[0, 7], [1, SD]]), op=MUL)

            def emit_vtab(j):
                # vtab[:, s, j, m] = px[m, s, j] * pq[15-m, s, j]
                eng = nc.gpsimd if j % 2 == 0 or j == 5 else nc.vector
                eng.tensor_tensor(
                    out=_ap(vtab[:, :, :, :], j * 16, [[1, 16], [DIM * 16, S]]),
                    in0=_ap(px[:, :, :], j, [[SD, 16], [DIM, S]]),
                    in1=_ap(pq[:, :, :], 15 * SD + j, [[-SD, 16], [DIM, S]]), op=MUL)

            # ---- stage D: cond -> XBAR transpose -> matmul -> stash ----
            def emit_group_mm(g, bb=None, gb=None):
                if bb is None:
                    bb = Bbig[:, :, :, :]
                    gb = g * NT * NJ * 4
                TS = NJ * 4
                k2g = condp.tile([P, NT, 16], F32, tag="k2")
                k3g = condp.tile([P, NT, 64], F32, tag="k3")
                q34g = condp.tile([P, NT, 16], F32, tag="q34")
                ea = nc.vector if g % 2 == 0 else nc.gpsimd
                eb = nc.gpsimd if g % 2 == 0 else nc.vector
                ea.tensor_tensor(
                    out=k2g[:, :, :],
                    in0=_ap(bb, gb + 0, [[TS, NT], [1, 4], [0, 4]]),
                    in1=_ap(bb, gb + 4, [[TS, NT], [0, 4], [1, 4]]), op=MUL)
                ea.tensor_tensor(
                    out=k3g[:, :, :],
                    in0=_ap(k2g[:, :, :], 0, [[16, NT], [1, 16], [0, 4]]),
                    in1=_ap(bb, gb + 8, [[TS, NT], [0, 16], [1, 4]]), op=MUL)
                eb.tensor_tensor(
                    out=q34g[:, :, :],
                    in0=_ap(bb, gb + 12, [[TS, NT], [1, 4], [0, 4]]),
                    in1=_ap(bb, gb + 16, [[TS, NT], [0, 4], [1, 4]]), op=MUL)
                ctb = ctbp.tile([P, KCH, NB], BF16, tag="ctb")
                for t in range(NT):
                    cnd = condp.tile([P, CDIM], BF16, tag=f"cond{t}")
                    idx = 4 * g + t
                    eng = nc.vector if (idx * 25) % 64 < 25 else nc.gpsimd
                    eng.tensor_tensor(
                        out=cnd[:, :],
                        in0=_ap(k3g[:, :, :], t * 64, [[1, 64], [0, 16]]),
                        in1=_ap(q34g[:, :, :], t * 16, [[0, 64], [1, 16]]), op=MUL)
                    if t > 0:
                        # XBAR: ctb[c%128, c//128, t*128+p] = cnd[p, c]
                        nc.sync.dma_start_transpose(
                            out=_ap(ctb[:, :, :], t * P, [[NB, KCH], [1, P]]),
                            in_=cnd[:, :])
                    else:
                        # PE transpose (8 blocks) + Act copy to offload XBAR
                        pct = ps_ct.tile([P, CDIM], BF16, tag="pct")
                        for k in range(KCH):
                            nc.tensor.matmul(
                                out=_ap(pct[:, :], k * P, [[1, P]]),
                                lhsT=cnd[:, k * P:(k + 1) * P],
                                rhs=idn[:, :],
                                is_transpose=True, start=True, stop=True,
                                skip_group_check=True)
                        nc.scalar.copy(
                            out=_ap(ctb[:, :, :], t * P, [[NB, KCH], [1, P]]),
                            in_=pct[:, :])
                tfps = ps_tf.tile([M96, NB], F32, tag="tf")
                for k in range(KCH):
                    nc.tensor.matmul(
                        out=tfps[:, :],
                        lhsT=wsb[:, k, :],
                        rhs=ctb[:, k, :],
                        start=(k == 0), stop=(k == KCH - 1))
                if g < NG - 2:
                    nc.scalar.copy(
                        out=_ap(tfsb[:, :, :], g * NB, [[1, NB]]),
                        in_=tfps[:, :])
                else:
                    nc.vector.tensor_copy(
                        out=_ap(tfsb[:, :, :], g * NB, [[1, NB]]),
                        in_=tfps[:, :])

            # ---- stage E: transposed combine per group ----
            def emit_group_combine(g):
                ppsi = ps_psi.tile([M96, NB], BF16, tag="ppsi")
                for t in range(NT):
                    nc.tensor.matmul(
                        out=_ap(ppsi[:, :], t * P, [[1, P]]),
                        lhsT=_ap(vtab[:, :, :, :], (4 * g + t) * M96, [[1, M96]]),
                        rhs=idn[:, :],
                        is_transpose=True, start=True, stop=True,
                        skip_group_check=True)
                psiT = eprodp.tile([M96, NB], BF16, tag="psiT")
                if g % 3 == 0:
                    nc.scalar.copy(out=psiT[:, :], in_=ppsi[:, :])
                else:
                    nc.vector.tensor_copy(out=psiT[:, :], in_=ppsi[:, :])
                eprod = eprodp.tile([M96, NB], BF16, tag="eprod")
                nc.gpsimd.tensor_tensor(
                    out=eprod[:, :],
                    in0=_ap(tfsb[:, :, :], g * NB, [[1, NB]]),
                    in1=psiT[:, :], op=MUL)
                fac = ps_fac.tile([DIM, NB], F32, tag="fac")
                nc.tensor.matmul(
                    out=fac[:, :], lhsT=e96[:, :], rhs=eprod[:, :],
                    start=True, stop=True)
                facsb = eprodp.tile([DIM, NB], F32R, tag="facsb")
                if g < NG - 3:
                    nc.scalar.copy(out=facsb[:, :], in_=fac[:, :])
                else:
                    nc.vector.tensor_copy(out=facsb[:, :], in_=fac[:, :])
                facT = ps_ft.tile([P, NT, DIM], F32R, tag="facT")
                for t in range(NT):
                    nc.tensor.matmul(
                        out=facT[:, t, :],
                        lhsT=_ap(facsb[:, :], t * P, [[1, P]]),
                        rhs=idnr[:DIM, :DIM],
                        is_transpose=True, start=True, stop=True,
                        skip_group_check=True)
                nc.scalar.copy(
                    out=_ap(facn[:, :, :], 4 * g * DIM, [[1, NT * DIM]]),
                    in_=facT[:, :, :])
                # incremental density product for this group's s-range
                eng3 = nc.vector if g % 2 == 0 else nc.gpsimd
                eng3.tensor_tensor(
                    out=_ap(t1f[:, :, :], 4 * g * 4, [[4, NT], [1, 3]]),
                    in0=_ap(facn[:, :, :], 4 * g * DIM, [[DIM, NT], [2, 3]]),
                    in1=_ap(facn[:, :, :], 4 * g * DIM + 1, [[DIM, NT], [2, 3]]),
                    op=MUL)
                eng3.tensor_tensor(
                    out=_ap(t1f[:, :, :], 4 * g * 4 + 3, [[4, NT]]),
                    in0=_ap(t1f[:, :, :], 4 * g * 4, [[4, NT]]),
                    in1=_ap(t1f[:, :, :], 4 * g * 4 + 1, [[4, NT]]), op=MUL)
                eng3.tensor_tensor(
                    out=_ap(dq[:, :], 4 * g, [[1, NT]]),
                    in0=_ap(t1f[:, :, :], 4 * g * 4 + 3, [[4, NT]]),
                    in1=_ap(t1f[:, :, :], 4 * g * 4 + 2, [[4, NT]]), op=MUL)
                if g == 7:
                    nc.sync.dma_start(out=dens_out[:, :32], in_=dq[:, :32])

            # ---- schedule: group-0 fast path, then full tables; ladders/vtab
            # interleaved into groups 1..6, combines trail mm by K groups ----
            K = 5
            emit_stages_ab()
            for g in range(0, NG):
                emit_group_mm(g)
                if g < 4:
                    ladder_chunk(g)
                elif g == 4:
                    emit_vtab(0), emit_vtab(1), emit_vtab(2)
                elif g == 5:
                    emit_vtab(3), emit_vtab(4), emit_vtab(5)
                if g >= K:
                    emit_group_combine(g - K)
            for g in range(NG - K, NG):
                emit_group_combine(g)

            # ---- final: second half of density already computed per group
            nc.sync.dma_start(out=dens_out[:, 32:], in_=dq[:, 32:])

    nc.finalize()
    return nc


def _softplus64(v):
    return np.logaddexp(0.0, v)


def _host_w(As):
    """W96 [1024, 96]: col i*16+m = kap_m * (ca_m - ca_{m-1}), ca_{-1}=0,
    ca_15=1 (binomial scaling + coeff finite-difference folded in)."""
    kap = 16.0 * np.array([math.comb(15, m) for m in range(16)], dtype=np.float64)
    cols = []
    for i in range(DIM):
        c = np.cumsum(_softplus64(As[i].astype(np.float64)), axis=1)
        ca = 2.0 * (1.0 / (1.0 + np.exp(-c)) - 0.5)
        rows = ca.shape[0]
        ext = np.concatenate(
            [np.zeros((rows, 1)), ca, np.ones((rows, 1))], axis=1)  # [r, 17]
        blk = kap * (ext[:, 1:] - ext[:, :-1])                      # [r, 16]
        cols.append(np.repeat(blk, 4 ** (5 - i), axis=0))
    return np.concatenate(cols, axis=1).astype(np.float32)


def kernel(**inputs):
    x = np.asarray(inputs["x"], dtype=np.float32)
    As = [np.asarray(inputs[f"A{i}"], dtype=np.float32) for i in range(DIM)]

    if "nc" not in _CACHE:
        _CACHE["nc"] = _build_nc()
    nc = _CACHE["nc"]

    w = _host_w(As)
    idn = np.eye(P, dtype=np.float32)
    e96 = np.zeros((M96, DIM), dtype=np.float32)
    for i in range(DIM):
        e96[i * 16:(i + 1) * 16, i] = 1.0

    in_maps = []
    for c in range(NCORES):
        xc = x[c * NC:(c + 1) * NC].reshape(P, S, DIM)
        in_maps.append({"xr": xc, "wmat": w, "ident": idn, "e96r": e96})

    res = run_bass_kernel_spmd(nc, in_maps, core_ids=list(range(NCORES)))
    outs = [r["dens"].reshape(NC) for r in res.results]
    return np.concatenate(outs, axis=0)


if __name__ == "__main__":
    rng = np.random.default_rng(0)
    ins = {"x": rng.uniform(0, 1, (N, DIM)).astype(np.float32)}
    for i in range(DIM):
        ins[f"A{i}"] = rng.uniform(0, 1, ((4 ** i), 15)).astype(np.float32)
    out = kernel(**ins)
    print(out.shape, out[:4])
